# revision 5
# baseline (speedup 1.0000x reference)
"""Trainium2 Bass kernel: GQA attention (B=2,T=2048,D=4096,N=32,K=8,H=128), fp32.

Sharding: tensor-parallel over heads across 8 cores. Core c owns q heads
[4c,4c+4) and kv head c. Each core computes its 4 heads' attention and a
partial output projection [B,T,D]; the host sums the 8 partials.

Per-core layout strategy:
  - x is fed transposed (xT [B,D,T]) so q/k projections run with d on
    partitions and produce qT/kT in [h, t] layout directly (fp32r matmuls).
  - RoPE applied on [h, t] psum tiles with host-precomputed cos/sin tables.
  - scores are computed transposed (PT [s, t] = (K^T)_s^T @ qT), masked via
    mult-by-0/1-mask after exp, so no max-subtraction or P transpose needed.
  - AV uses PT tiles as stationary lhsT with V (+ones column) [s, h|1] bf16:
    out psum [t, 128+1] gives both the weighted sum and the softmax
    denominator; eviction normalizes via per-partition reciprocal scale.
  - o-proj: out tiles PE-transposed to [h, t], then lhsT=oT [h,t] x
    rhs=wo [h,d] accumulated over the 4 heads -> partial [t, d].
"""

import numpy as np

B, T, D, NH, KH, H = 2, 2048, 4096, 32, 8, 128
NC = 8
G = NH // NC          # q heads per core = 4
TC = 512              # t-chunk
NTC = T // TC         # 4
ST = 128              # s-tile
NST = T // ST         # 16
ND = D // 128         # 32 d-tiles
SCALE = float(H) ** -0.5
ROPE_THETA = 500000.0

_CACHE = {}


def _classify(attn_mask):
    """cls[b][tc][si] in {0:zero, 1:full, 2:partial} from mask[b,t,s]."""
    cls = []
    for b in range(B):
        per_tc = []
        for tc in range(NTC):
            row = []
            for si in range(NST):
                blk = attn_mask[b, tc * TC:(tc + 1) * TC, si * ST:(si + 1) * ST]
                if not blk.any():
                    row.append(0)
                elif blk.all():
                    row.append(1)
                else:
                    row.append(2)
            per_tc.append(row)
        cls.append(per_tc)
    return cls


def _build(cls):
    import concourse.tile as tile
    from concourse import bacc, mybir
    from concourse.masks import make_identity

    f32 = mybir.dt.float32
    f32r = mybir.dt.float32r
    bf16 = mybir.dt.bfloat16
    AF = mybir.ActivationFunctionType

    nc = bacc.Bacc(None)
    xT = nc.declare_dram_parameter("xT", [B, D, T], f32r, isOutput=False)
    cosT = nc.declare_dram_parameter("cosT", [B, 64, T], f32, isOutput=False)
    sinT = nc.declare_dram_parameter("sinT", [B, 64, T], f32, isOutput=False)
    maskT = nc.declare_dram_parameter("maskT", [B, T, T], bf16, isOutput=False)
    wq_c = nc.declare_dram_parameter("wq_c", [G, D, H], f32r, isOutput=False)
    wk_c = nc.declare_dram_parameter("wk_c", [D, H], f32r, isOutput=False)
    wv_c = nc.declare_dram_parameter("wv_c", [D, H], bf16, isOutput=False)
    wo_c = nc.declare_dram_parameter("wo_c", [G, H, D], bf16, isOutput=False)
    pout = nc.declare_dram_parameter("pout", [B, T, D], f32, isOutput=True)

    with tile.TileContext(nc) as tc_:
        with (
            tc_.tile_pool(name="const", bufs=1) as const,
            tc_.tile_pool(name="wpool", bufs=1) as wpool,
            tc_.tile_pool(name="perb", bufs=1) as perb,
            tc_.tile_pool(name="qp", bufs=1) as qp,
            tc_.tile_pool(name="xs", bufs=2) as xs,
            tc_.tile_pool(name="pt", bufs=1) as ptp,
            tc_.tile_pool(name="mk", bufs=2) as mkp,
            tc_.tile_pool(name="rp", bufs=2) as rp,
            tc_.tile_pool(name="sm", bufs=4) as sm,
            tc_.tile_pool(name="op", bufs=1) as op,
            tc_.tile_pool(name="obp", bufs=2) as obp,
            tc_.tile_pool(name="wop", bufs=2) as wop,
            tc_.tile_pool(name="ps", bufs=1, space="PSUM") as ps,
        ):
            ident_b = const.tile([128, 128], bf16)
            make_identity(nc, ident_b[:])

            # resident weights (wq f32, wk f32, wv bf16); wo is streamed
            wq_sb = []
            for n in range(G):
                t = wpool.tile([128, ND, H], f32r, tag=f"wq{n}", name=f"wq{n}")
                nc.sync.dma_start(
                    out=t[:], in_=wq_c[n].rearrange("(a p) h -> p a h", p=128))
                wq_sb.append(t)
            wk_sb = wpool.tile([128, ND, H], f32r, tag="wk")
            nc.sync.dma_start(
                out=wk_sb[:], in_=wk_c.rearrange("(a p) h -> p a h", p=128))
            wv_sb = wpool.tile([128, ND, H], bf16, tag="wv")
            nc.sync.dma_start(
                out=wv_sb[:], in_=wv_c.rearrange("(a p) h -> p a h", p=128))

            for b in range(B):
                cssn = perb.tile([128, T], f32, tag="cssn")
                nc.sync.dma_start(out=cssn[0:64, :], in_=cosT[b])
                nc.sync.dma_start(out=cssn[64:128, :], in_=sinT[b])
                kT_sb = perb.tile([128, T], f32r, tag="kT")
                v_sb = [perb.tile([128, H + 1], bf16, tag=f"v{si}",
                                  name=f"v{si}") for si in range(NST)]
                for si in range(NST):
                    nc.vector.memset(v_sb[si][:, H:H + 1], 1.0)

                for tcx in range(NTC):
                    tsl = slice(tcx * TC, (tcx + 1) * TC)
                    # ---- projections for this t-chunk ----
                    qps = [ps.tile([128, TC], f32, tag=f"qps{n}",
                                   name=f"qps{n}") for n in range(G)]
                    kps = ps.tile([128, TC], f32, tag="kps")
                    vps = ps.tile([128, TC], f32, tag="vps")
                    for di in range(ND):
                        xt = xs.tile([128, TC], f32r, tag="xt")
                        nc.sync.dma_start(
                            out=xt[:], in_=xT[b, di * 128:(di + 1) * 128, tsl])
                        xtb = xs.tile([128, TC], bf16, tag="xtb")
                        nc.vector.tensor_copy(out=xtb[:], in_=xt[:])
                        st, sp = di == 0, di == ND - 1
                        for n in range(G):
                            nc.tensor.matmul(
                                qps[n][:], wq_sb[n][:, di, :],
                                xt[:], start=st, stop=sp)
                        nc.tensor.matmul(
                            kps[:], wk_sb[:, di, :],
                            xt[:], start=st, stop=sp)
                        nc.tensor.matmul(
                            vps[:], wv_sb[:, di, :], xtb[:], start=st, stop=sp)

                    # ---- RoPE eviction: psum [h, t] -> sbuf ----
                    cs, sn = cssn[0:64, tsl], cssn[64:128, tsl]
                    qT = []
                    for n in range(G):
                        qt = qp.tile([128, TC], f32r, tag=f"q{n}", name=f"q{n}")
                        t1 = rp.tile([64, TC], f32, tag="r1")
                        t2 = rp.tile([64, TC], f32, tag="r2")
                        nc.vector.tensor_mul(t1[:], qps[n][0:64, :], cs)
                        nc.vector.tensor_mul(t2[:], qps[n][64:128, :], sn)
                        nc.vector.tensor_sub(qt[0:64, :], t1[:], t2[:])
                        t3 = rp.tile([64, TC], f32, tag="r3")
                        t4 = rp.tile([64, TC], f32, tag="r4")
                        nc.vector.tensor_mul(t3[:], qps[n][64:128, :], cs)
                        nc.vector.tensor_mul(t4[:], qps[n][0:64, :], sn)
                        nc.vector.tensor_add(qt[64:128, :], t3[:], t4[:])
                        qT.append(qt)
                    t1 = rp.tile([64, TC], f32, tag="r1")
                    t2 = rp.tile([64, TC], f32, tag="r2")
                    nc.vector.tensor_mul(t1[:], kps[0:64, :], cs)
                    nc.vector.tensor_mul(t2[:], kps[64:128, :], sn)
                    nc.vector.tensor_sub(kT_sb[0:64, tsl], t1[:], t2[:])
                    t3 = rp.tile([64, TC], f32, tag="r3")
                    t4 = rp.tile([64, TC], f32, tag="r4")
                    nc.vector.tensor_mul(t3[:], kps[64:128, :], cs)
                    nc.vector.tensor_mul(t4[:], kps[0:64, :], sn)
                    nc.vector.tensor_add(kT_sb[64:128, tsl], t3[:], t4[:])
                    # v: cast + transpose to [s, h] bf16
                    vb = rp.tile([128, TC], bf16, tag="vb")
                    nc.vector.tensor_copy(out=vb[:], in_=vps[:])
                    for j in range(TC // 128):
                        vtp = ps.tile([128, 128], bf16, tag="vps", name="vtp")
                        nc.tensor.transpose(
                            vtp[:], vb[:, j * 128:(j + 1) * 128], ident_b[:])
                        nc.vector.tensor_copy(
                            out=v_sb[tcx * 4 + j][:, 0:H], in_=vtp[:])

                    # ---- attention for this t-chunk ----
                    slist = [si for si in range(NST) if cls[b][tcx][si] != 0]
                    oT = [[None] * (TC // 128) for _ in range(G)]
                    for n in range(G):
                        pts = {}
                        for ii, si in enumerate(slist):
                            pps = ps.tile([128, TC], f32,
                                          tag=f"qps{ii % 2}", name="pps")
                            nc.tensor.matmul(
                                pps[:],
                                kT_sb[:, si * ST:(si + 1) * ST],
                                qT[n][:], start=True, stop=True)
                            ptt = ptp.tile([128, TC], bf16, tag=f"pt{si}",
                                           name=f"pt{si}")
                            nc.scalar.activation(
                                ptt[:], pps[:], AF.Exp, scale=SCALE)
                            if cls[b][tcx][si] == 2:
                                mt = mkp.tile([128, TC], bf16, tag="mk")
                                nc.sync.dma_start(
                                    out=mt[:],
                                    in_=maskT[b, si * ST:(si + 1) * ST, tsl])
                                nc.vector.tensor_mul(ptt[:], ptt[:], mt[:])
                            pts[si] = ptt
                        for ts in range(TC // 128):
                            avp = ps.tile([128, H + 1], f32,
                                          tag=f"qps{2 + ts % 2}", name="avp")
                            for i, si in enumerate(slist):
                                nc.tensor.matmul(
                                    avp[:],
                                    pts[si][:, ts * 128:(ts + 1) * 128],
                                    v_sb[si][:], start=i == 0,
                                    stop=i == len(slist) - 1)
                            rcp = sm.tile([128, 1], f32, tag="rcp")
                            nc.vector.reciprocal(rcp[:], avp[:, H:H + 1])
                            osb = sm.tile([128, 128], bf16, tag="osb")
                            nc.scalar.activation(
                                osb[:], avp[:, 0:H], AF.Copy, scale=rcp[:])
                            otp = ps.tile([128, 128], bf16, tag="kps",
                                          name="otp")
                            nc.tensor.transpose(otp[:], osb[:], ident_b[:])
                            ot = op.tile([128, 128], bf16, tag=f"oT{n}_{ts}",
                                         name=f"oT{n}_{ts}")
                            nc.vector.tensor_copy(out=ot[:], in_=otp[:])
                            oT[n][ts] = ot

                    # ---- o-proj for this t-chunk (wo streamed per dc) ----
                    for dc in range(D // TC):
                        wo_t = []
                        for n in range(G):
                            wt = wop.tile([128, TC], bf16, tag=f"wo{n}",
                                          name=f"wo{n}")
                            nc.sync.dma_start(
                                out=wt[:],
                                in_=wo_c[n][:, dc * TC:(dc + 1) * TC])
                            wo_t.append(wt)
                        for ts in range(TC // 128):
                            ops = ps.tile([128, TC], f32,
                                          tag=("vps", "kps")[dc % 2],
                                          name="ops")
                            for n in range(G):
                                nc.tensor.matmul(
                                    ops[:], oT[n][ts][:], wo_t[n][:],
                                    start=n == 0, stop=n == G - 1)
                            ob = obp.tile([128, TC], f32, tag="ob")
                            nc.vector.tensor_copy(out=ob[:], in_=ops[:])
                            trow = tcx * TC + ts * 128
                            nc.sync.dma_start(
                                out=pout[b, trow:trow + 128,
                                         dc * TC:(dc + 1) * TC],
                                in_=ob[:])
    nc.finalize()
    return nc


def kernel(x, segment_pos, attn_mask, wq, wk, wv, wo):
    import ml_dtypes
    from concourse.bass_utils import run_bass_kernel_spmd

    x = np.asarray(x, dtype=np.float32)
    attn_mask = np.asarray(attn_mask).astype(bool)
    bf = ml_dtypes.bfloat16

    # host prep
    xT = np.ascontiguousarray(x.transpose(0, 2, 1))
    pos = np.asarray(segment_pos).astype(np.float32)
    fraction = (2.0 * np.arange(64, dtype=np.float32)) / float(H)
    timescale = (ROPE_THETA ** fraction).astype(np.float32)
    sinusoid = pos[:, :, None] / timescale[None, None, :]  # [B,T,64]
    cosT = np.ascontiguousarray(
        np.cos(sinusoid).astype(np.float32).transpose(0, 2, 1))
    sinT = np.ascontiguousarray(
        np.sin(sinusoid).astype(np.float32).transpose(0, 2, 1))
    maskT = np.ascontiguousarray(
        attn_mask.transpose(0, 2, 1)).astype(bf)
    wq_r = np.ascontiguousarray(
        np.asarray(wq, dtype=np.float32).transpose(1, 0, 2))  # [N,D,H]
    wk_r = np.ascontiguousarray(
        np.asarray(wk, dtype=np.float32).transpose(1, 0, 2))  # [K,D,H]
    wv_r = np.ascontiguousarray(
        np.asarray(wv, dtype=np.float32).transpose(1, 0, 2)).astype(bf)
    wo_b = np.asarray(wo, dtype=np.float32).astype(bf)       # [N,H,D]

    cls = _classify(attn_mask)
    key = str(cls)
    if key not in _CACHE:
        _CACHE[key] = _build(cls)
    nc = _CACHE[key]

    in_maps = []
    for c in range(NC):
        in_maps.append({
            "xT": xT, "cosT": cosT, "sinT": sinT, "maskT": maskT,
            "wq_c": np.ascontiguousarray(wq_r[G * c:G * (c + 1)]),
            "wk_c": np.ascontiguousarray(wk_r[c]),
            "wv_c": np.ascontiguousarray(wv_r[c]),
            "wo_c": np.ascontiguousarray(wo_b[G * c:G * (c + 1)]),
        })
    res = run_bass_kernel_spmd(nc, in_maps, list(range(NC)))
    out = res.results[0]["pout"].astype(np.float32)
    for c in range(1, NC):
        out += res.results[c]["pout"]
    return out


# revision 6
# speedup vs baseline: 2.1771x; 2.1771x over previous
"""Trainium2 Bass kernel: GQA attention (B=2,T=2048,D=4096,N=32,K=8,H=128), fp32.

Sharding: tensor-parallel over heads across 8 cores. Core c owns q heads
[4c,4c+4) and kv head c. Each core computes its 4 heads' attention and a
partial output projection [B,T,D]; the host sums the 8 partials.

Per-core layout strategy:
  - x is fed transposed (xT [B,D,T]) so q/k projections run with d on
    partitions and produce qT/kT in [h, t] layout directly (fp32r matmuls).
  - RoPE applied on [h, t] psum tiles with host-precomputed cos/sin tables.
  - scores are computed transposed (PT [s, t] = (K^T)_s^T @ qT), masked via
    mult-by-0/1-mask after exp, so no max-subtraction or P transpose needed.
  - AV uses PT tiles as stationary lhsT with V (+ones column) [s, h|1] bf16:
    out psum [t, 128+1] gives both the weighted sum and the softmax
    denominator; eviction normalizes via per-partition reciprocal scale.
  - o-proj: out tiles PE-transposed to [h, t], then lhsT=oT [h,t] x
    rhs=wo [h,d] accumulated over the 4 heads -> partial [t, d].
"""

import numpy as np

B, T, D, NH, KH, H = 2, 2048, 4096, 32, 8, 128
NC = 8
G = NH // NC          # q heads per core = 4
TC = 512              # t-chunk
NTC = T // TC         # 4
ST = 128              # s-tile
NST = T // ST         # 16
ND = D // 128         # 32 d-tiles
SCALE = float(H) ** -0.5
ROPE_THETA = 500000.0

_CACHE = {}


def _classify(attn_mask):
    """cls[b][tc][si] in {0:zero, 1:full, 2:partial} from mask[b,t,s]."""
    cls = []
    for b in range(B):
        per_tc = []
        for tc in range(NTC):
            row = []
            for si in range(NST):
                blk = attn_mask[b, tc * TC:(tc + 1) * TC, si * ST:(si + 1) * ST]
                if not blk.any():
                    row.append(0)
                elif blk.all():
                    row.append(1)
                else:
                    row.append(2)
            per_tc.append(row)
        cls.append(per_tc)
    return cls


def _build(cls):
    import concourse.tile as tile
    from concourse import bacc, mybir
    from concourse.masks import make_identity

    f32 = mybir.dt.float32
    f32r = mybir.dt.float32r
    bf16 = mybir.dt.bfloat16
    AF = mybir.ActivationFunctionType

    nc = bacc.Bacc(None)
    xT = nc.declare_dram_parameter("xT", [B, D, T], f32r, isOutput=False)
    cosT = nc.declare_dram_parameter("cosT", [B, 64, T], f32, isOutput=False)
    sinT = nc.declare_dram_parameter("sinT", [B, 64, T], f32, isOutput=False)
    parts = [(b, tcx, si) for b in range(B) for tcx in range(NTC)
             for si in range(NST) if cls[b][tcx][si] == 2]
    pidx = {k: i for i, k in enumerate(parts)}
    maskP = nc.declare_dram_parameter(
        "maskP", [max(1, len(parts)), ST, TC], bf16, isOutput=False)
    wq_c = nc.declare_dram_parameter("wq_c", [G, D, H], f32r, isOutput=False)
    wk_c = nc.declare_dram_parameter("wk_c", [D, H], f32r, isOutput=False)
    wv_c = nc.declare_dram_parameter("wv_c", [D, H], bf16, isOutput=False)
    wo_c = nc.declare_dram_parameter("wo_c", [G, H, D], bf16, isOutput=False)
    pout = nc.declare_dram_parameter("pout", [B * T // NC, D], f32, isOutput=True)

    with tile.TileContext(nc) as tc_:
        with (
            tc_.tile_pool(name="const", bufs=1) as const,
            tc_.tile_pool(name="wpool", bufs=1) as wpool,
            tc_.tile_pool(name="perb", bufs=1) as perb,
            tc_.tile_pool(name="qp", bufs=1) as qp,
            tc_.tile_pool(name="xs", bufs=2) as xs,
            tc_.tile_pool(name="pt", bufs=1) as ptp,
            tc_.tile_pool(name="mk", bufs=2) as mkp,
            tc_.tile_pool(name="rp", bufs=2) as rp,
            tc_.tile_pool(name="sm", bufs=4) as sm,
            tc_.tile_pool(name="op", bufs=1) as op,
            tc_.tile_pool(name="obp", bufs=2) as obp,
            tc_.tile_pool(name="wop", bufs=2) as wop,
            tc_.tile_pool(name="ps", bufs=1, space="PSUM") as ps,
            tc_.tile_pool(name="dram", bufs=1, space="DRAM") as dram,
        ):
            pout_i = dram.tile([B * T, D], f32)
            rs_out = dram.tile([B * T // NC, D], f32)
            ident_b = const.tile([128, 128], bf16)
            make_identity(nc, ident_b[:])

            # resident weights (wq f32, wk f32, wv bf16); wo is streamed
            wq_sb = []
            for n in range(G):
                t = wpool.tile([128, ND, H], f32r, tag=f"wq{n}", name=f"wq{n}")
                nc.sync.dma_start(
                    out=t[:], in_=wq_c[n].rearrange("(a p) h -> p a h", p=128))
                wq_sb.append(t)
            wk_sb = wpool.tile([128, ND, H], f32r, tag="wk")
            nc.sync.dma_start(
                out=wk_sb[:], in_=wk_c.rearrange("(a p) h -> p a h", p=128))
            wv_sb = wpool.tile([128, ND, H], bf16, tag="wv")
            nc.sync.dma_start(
                out=wv_sb[:], in_=wv_c.rearrange("(a p) h -> p a h", p=128))

            for b in range(B):
                cssn = perb.tile([128, T], f32, tag="cssn")
                nc.sync.dma_start(out=cssn[0:64, :], in_=cosT[b])
                nc.sync.dma_start(out=cssn[64:128, :], in_=sinT[b])
                kT_sb = perb.tile([128, T], f32r, tag="kT")
                v_sb = [perb.tile([128, H + 1], bf16, tag=f"v{si}",
                                  name=f"v{si}") for si in range(NST)]
                for si in range(NST):
                    nc.vector.memset(v_sb[si][:, H:H + 1], 1.0)

                for tcx in range(NTC):
                    tsl = slice(tcx * TC, (tcx + 1) * TC)
                    # ---- projections for this t-chunk ----
                    qps = [ps.tile([128, TC], f32, tag=f"qps{n}",
                                   name=f"qps{n}") for n in range(G)]
                    kps = ps.tile([128, TC], f32, tag="kps")
                    vps = ps.tile([128, TC], f32, tag="vps")
                    for di in range(ND):
                        xt = xs.tile([128, TC], f32r, tag="xt")
                        nc.sync.dma_start(
                            out=xt[:], in_=xT[b, di * 128:(di + 1) * 128, tsl])
                        xtb = xs.tile([128, TC], bf16, tag="xtb")
                        nc.vector.tensor_copy(out=xtb[:], in_=xt[:])
                        st, sp = di == 0, di == ND - 1
                        for n in range(G):
                            nc.tensor.matmul(
                                qps[n][:], wq_sb[n][:, di, :],
                                xt[:], start=st, stop=sp)
                        nc.tensor.matmul(
                            kps[:], wk_sb[:, di, :],
                            xt[:], start=st, stop=sp)
                        nc.tensor.matmul(
                            vps[:], wv_sb[:, di, :], xtb[:], start=st, stop=sp)

                    # ---- RoPE eviction: psum [h, t] -> sbuf ----
                    cs, sn = cssn[0:64, tsl], cssn[64:128, tsl]
                    qT = []
                    for n in range(G):
                        qt = qp.tile([128, TC], f32r, tag=f"q{n}", name=f"q{n}")
                        t1 = rp.tile([64, TC], f32, tag="r1")
                        t2 = rp.tile([64, TC], f32, tag="r2")
                        nc.vector.tensor_mul(t1[:], qps[n][0:64, :], cs)
                        nc.vector.tensor_mul(t2[:], qps[n][64:128, :], sn)
                        nc.vector.tensor_sub(qt[0:64, :], t1[:], t2[:])
                        t3 = rp.tile([64, TC], f32, tag="r3")
                        t4 = rp.tile([64, TC], f32, tag="r4")
                        nc.vector.tensor_mul(t3[:], qps[n][64:128, :], cs)
                        nc.vector.tensor_mul(t4[:], qps[n][0:64, :], sn)
                        nc.vector.tensor_add(qt[64:128, :], t3[:], t4[:])
                        qT.append(qt)
                    t1 = rp.tile([64, TC], f32, tag="r1")
                    t2 = rp.tile([64, TC], f32, tag="r2")
                    nc.vector.tensor_mul(t1[:], kps[0:64, :], cs)
                    nc.vector.tensor_mul(t2[:], kps[64:128, :], sn)
                    nc.vector.tensor_sub(kT_sb[0:64, tsl], t1[:], t2[:])
                    t3 = rp.tile([64, TC], f32, tag="r3")
                    t4 = rp.tile([64, TC], f32, tag="r4")
                    nc.vector.tensor_mul(t3[:], kps[64:128, :], cs)
                    nc.vector.tensor_mul(t4[:], kps[0:64, :], sn)
                    nc.vector.tensor_add(kT_sb[64:128, tsl], t3[:], t4[:])
                    # v: cast + transpose to [s, h] bf16
                    vb = rp.tile([128, TC], bf16, tag="vb")
                    nc.vector.tensor_copy(out=vb[:], in_=vps[:])
                    for j in range(TC // 128):
                        vtp = ps.tile([128, 128], bf16, tag="vps", name="vtp")
                        nc.tensor.transpose(
                            vtp[:], vb[:, j * 128:(j + 1) * 128], ident_b[:])
                        nc.vector.tensor_copy(
                            out=v_sb[tcx * 4 + j][:, 0:H], in_=vtp[:])

                    # ---- attention for this t-chunk ----
                    slist = [si for si in range(NST) if cls[b][tcx][si] != 0]
                    oT = [[None] * (TC // 128) for _ in range(G)]
                    for n in range(G):
                        pts = {}
                        for ii, si in enumerate(slist):
                            pps = ps.tile([128, TC], f32,
                                          tag=f"qps{ii % 2}", name="pps")
                            nc.tensor.matmul(
                                pps[:],
                                kT_sb[:, si * ST:(si + 1) * ST],
                                qT[n][:], start=True, stop=True)
                            ptt = ptp.tile([128, TC], bf16, tag=f"pt{si}",
                                           name=f"pt{si}")
                            nc.scalar.activation(
                                ptt[:], pps[:], AF.Exp, scale=SCALE)
                            if cls[b][tcx][si] == 2:
                                mt = mkp.tile([128, TC], bf16, tag="mk")
                                nc.sync.dma_start(
                                    out=mt[:],
                                    in_=maskP[pidx[(b, tcx, si)]])
                                nc.vector.tensor_mul(ptt[:], ptt[:], mt[:])
                            pts[si] = ptt
                        for ts in range(TC // 128):
                            avp = ps.tile([128, H + 1], f32,
                                          tag=f"qps{2 + ts % 2}", name="avp")
                            for i, si in enumerate(slist):
                                nc.tensor.matmul(
                                    avp[:],
                                    pts[si][:, ts * 128:(ts + 1) * 128],
                                    v_sb[si][:], start=i == 0,
                                    stop=i == len(slist) - 1)
                            rcp = sm.tile([128, 1], f32, tag="rcp")
                            nc.vector.reciprocal(rcp[:], avp[:, H:H + 1])
                            osb = sm.tile([128, 128], bf16, tag="osb")
                            nc.scalar.activation(
                                osb[:], avp[:, 0:H], AF.Copy, scale=rcp[:])
                            otp = ps.tile([128, 128], bf16, tag="kps",
                                          name="otp")
                            nc.tensor.transpose(otp[:], osb[:], ident_b[:])
                            ot = op.tile([128, 128], bf16, tag=f"oT{n}_{ts}",
                                         name=f"oT{n}_{ts}")
                            nc.vector.tensor_copy(out=ot[:], in_=otp[:])
                            oT[n][ts] = ot

                    # ---- o-proj for this t-chunk (wo streamed per dc) ----
                    for dc in range(D // TC):
                        wo_t = []
                        for n in range(G):
                            wt = wop.tile([128, TC], bf16, tag=f"wo{n}",
                                          name=f"wo{n}")
                            nc.sync.dma_start(
                                out=wt[:],
                                in_=wo_c[n][:, dc * TC:(dc + 1) * TC])
                            wo_t.append(wt)
                        for ts in range(TC // 128):
                            ops = ps.tile([128, TC], f32,
                                          tag=("vps", "kps")[dc % 2],
                                          name="ops")
                            for n in range(G):
                                nc.tensor.matmul(
                                    ops[:], oT[n][ts][:], wo_t[n][:],
                                    start=n == 0, stop=n == G - 1)
                            ob = obp.tile([128, TC], f32, tag="ob")
                            nc.vector.tensor_copy(out=ob[:], in_=ops[:])
                            trow = tcx * TC + ts * 128
                            nc.sync.dma_start(
                                out=pout_i[b * T + trow:b * T + trow + 128,
                                           dc * TC:(dc + 1) * TC],
                                in_=ob[:])
            nc.gpsimd.collective_compute(
                "ReduceScatter", mybir.AluOpType.add,
                replica_groups=[list(range(NC))],
                ins=[pout_i.opt()], outs=[rs_out.opt()])
            nc.sync.dma_start(out=pout[:, :], in_=rs_out[:])
    nc.finalize()
    return nc


def kernel(x, segment_pos, attn_mask, wq, wk, wv, wo):
    import ml_dtypes
    from concourse.bass_utils import run_bass_kernel_spmd

    x = np.asarray(x, dtype=np.float32)
    attn_mask = np.asarray(attn_mask).astype(bool)
    bf = ml_dtypes.bfloat16

    # host prep
    xT = np.ascontiguousarray(x.transpose(0, 2, 1))
    pos = np.asarray(segment_pos).astype(np.float32)
    fraction = (2.0 * np.arange(64, dtype=np.float32)) / float(H)
    timescale = (ROPE_THETA ** fraction).astype(np.float32)
    sinusoid = pos[:, :, None] / timescale[None, None, :]  # [B,T,64]
    cosT = np.ascontiguousarray(
        np.cos(sinusoid).astype(np.float32).transpose(0, 2, 1))
    sinT = np.ascontiguousarray(
        np.sin(sinusoid).astype(np.float32).transpose(0, 2, 1))
    cls = _classify(attn_mask)
    parts = [(b, tcx, si) for b in range(B) for tcx in range(NTC)
             for si in range(NST) if cls[b][tcx][si] == 2]
    if parts:
        maskP = np.stack([
            np.ascontiguousarray(
                attn_mask[b, tcx * TC:(tcx + 1) * TC,
                          si * ST:(si + 1) * ST].T).astype(bf)
            for (b, tcx, si) in parts])
    else:
        maskP = np.zeros((1, ST, TC), dtype=bf)
    wq_r = np.ascontiguousarray(
        np.asarray(wq, dtype=np.float32).transpose(1, 0, 2))  # [N,D,H]
    wk_r = np.ascontiguousarray(
        np.asarray(wk, dtype=np.float32).transpose(1, 0, 2))  # [K,D,H]
    wv_r = np.ascontiguousarray(
        np.asarray(wv, dtype=np.float32).transpose(1, 0, 2)).astype(bf)
    wo_b = np.asarray(wo, dtype=np.float32).astype(bf)       # [N,H,D]

    key = str(cls)
    if key not in _CACHE:
        _CACHE[key] = _build(cls)
    nc = _CACHE[key]

    in_maps = []
    for c in range(NC):
        in_maps.append({
            "xT": xT, "cosT": cosT, "sinT": sinT, "maskP": maskP,
            "wq_c": np.ascontiguousarray(wq_r[G * c:G * (c + 1)]),
            "wk_c": np.ascontiguousarray(wk_r[c]),
            "wv_c": np.ascontiguousarray(wv_r[c]),
            "wo_c": np.ascontiguousarray(wo_b[G * c:G * (c + 1)]),
        })
    res = run_bass_kernel_spmd(nc, in_maps, list(range(NC)))
    out = np.concatenate([res.results[c]["pout"] for c in range(NC)], axis=0)
    return np.ascontiguousarray(out.reshape(B, T, D).astype(np.float32))


# revision 7
# speedup vs baseline: 5.2005x; 2.3887x over previous
"""Trainium2 Bass kernel: GQA attention (B=2,T=2048,D=4096,N=32,K=8,H=128), fp32.

Sharding: tensor-parallel over heads across 8 cores. Core c owns q heads
[4c,4c+4) and kv head c. Each core computes its 4 heads' attention and a
partial output projection [B,T,D]; the host sums the 8 partials.

Per-core layout strategy:
  - x is fed transposed (xT [B,D,T]) so q/k projections run with d on
    partitions and produce qT/kT in [h, t] layout directly (fp32r matmuls).
  - RoPE applied on [h, t] psum tiles with host-precomputed cos/sin tables.
  - scores are computed transposed (PT [s, t] = (K^T)_s^T @ qT), masked via
    mult-by-0/1-mask after exp, so no max-subtraction or P transpose needed.
  - AV uses PT tiles as stationary lhsT with V (+ones column) [s, h|1] bf16:
    out psum [t, 128+1] gives both the weighted sum and the softmax
    denominator; eviction normalizes via per-partition reciprocal scale.
  - o-proj: out tiles PE-transposed to [h, t], then lhsT=oT [h,t] x
    rhs=wo [h,d] accumulated over the 4 heads -> partial [t, d].
"""

import numpy as np

B, T, D, NH, KH, H = 2, 2048, 4096, 32, 8, 128
NC = 8
G = NH // NC          # q heads per core = 4
TC = 512              # t-chunk
NTC = T // TC         # 4
ST = 128              # s-tile
NST = T // ST         # 16
ND = D // 128         # 32 d-tiles
SCALE = float(H) ** -0.5
ROPE_THETA = 500000.0

_CACHE = {}


def _classify(attn_mask):
    """cls[b][tc][si] in {0:zero, 1:full, 2:partial} from mask[b,t,s]."""
    cls = []
    for b in range(B):
        per_tc = []
        for tc in range(NTC):
            row = []
            for si in range(NST):
                blk = attn_mask[b, tc * TC:(tc + 1) * TC, si * ST:(si + 1) * ST]
                if not blk.any():
                    row.append(0)
                elif blk.all():
                    row.append(1)
                else:
                    row.append(2)
            per_tc.append(row)
        cls.append(per_tc)
    return cls


def _build(cls):
    import concourse.tile as tile
    from concourse import bacc, mybir
    from concourse.masks import make_identity

    f32 = mybir.dt.float32
    f32r = mybir.dt.float32r
    bf16 = mybir.dt.bfloat16
    AF = mybir.ActivationFunctionType

    nc = bacc.Bacc(None)
    xsl = nc.declare_dram_parameter("xsl", [B, D // NC, T], f32r, isOutput=False)
    cosT = nc.declare_dram_parameter("cosT", [B, 64, T], f32, isOutput=False)
    sinT = nc.declare_dram_parameter("sinT", [B, 64, T], f32, isOutput=False)
    parts = [(b, tcx, si) for b in range(B) for tcx in range(NTC)
             for si in range(NST) if cls[b][tcx][si] == 2]
    pidx = {k: i for i, k in enumerate(parts)}
    maskP = nc.declare_dram_parameter(
        "maskP", [max(1, len(parts)), ST, TC], bf16, isOutput=False)
    wq_c = nc.declare_dram_parameter("wq_c", [G, D, H], f32r, isOutput=False)
    wk_c = nc.declare_dram_parameter("wk_c", [D, H], f32r, isOutput=False)
    wv_c = nc.declare_dram_parameter("wv_c", [D, H], bf16, isOutput=False)
    wo_c = nc.declare_dram_parameter("wo_c", [G, H, D], bf16, isOutput=False)
    pout = nc.declare_dram_parameter("pout", [B * T // NC, D], f32, isOutput=True)

    with tile.TileContext(nc) as tc_:
        with (
            tc_.tile_pool(name="const", bufs=1) as const,
            tc_.tile_pool(name="wpool", bufs=1) as wpool,
            tc_.tile_pool(name="perb", bufs=1) as perb,
            tc_.tile_pool(name="qp", bufs=1) as qp,
            tc_.tile_pool(name="xs", bufs=2) as xs,
            tc_.tile_pool(name="pt", bufs=1) as ptp,
            tc_.tile_pool(name="mk", bufs=2) as mkp,
            tc_.tile_pool(name="rp", bufs=2) as rp,
            tc_.tile_pool(name="sm", bufs=4) as sm,
            tc_.tile_pool(name="op", bufs=1) as op,
            tc_.tile_pool(name="obp", bufs=2) as obp,
            tc_.tile_pool(name="wop", bufs=2) as wop,
            tc_.tile_pool(name="ps", bufs=1, space="PSUM") as ps,
            tc_.tile_pool(name="dram", bufs=1, space="DRAM") as dram,
        ):
            pout_i = dram.tile([B * T, D], f32)
            rs_out = dram.tile([B * T // NC, D], f32)
            xbounce = dram.tile([B, D // NC, T], f32r)
            xg = dram.tile([NC * B, D // NC, T], f32r, addr_space="Shared")
            nc.sync.dma_start(out=xbounce[:], in_=xsl[:, :, :])
            nc.gpsimd.collective_compute(
                "AllGather", mybir.AluOpType.bypass,
                replica_groups=[list(range(NC))],
                ins=[xbounce.opt()], outs=[xg.opt()])
            ident_b = const.tile([128, 128], bf16)
            make_identity(nc, ident_b[:])

            # resident weights (wq f32, wk f32, wv bf16); wo is streamed
            wq_sb = []
            for n in range(G):
                t = wpool.tile([128, ND, H], f32r, tag=f"wq{n}", name=f"wq{n}")
                nc.sync.dma_start(
                    out=t[:], in_=wq_c[n].rearrange("(a p) h -> p a h", p=128))
                wq_sb.append(t)
            wk_sb = wpool.tile([128, ND, H], f32r, tag="wk")
            nc.sync.dma_start(
                out=wk_sb[:], in_=wk_c.rearrange("(a p) h -> p a h", p=128))
            wv_sb = wpool.tile([128, ND, H], bf16, tag="wv")
            nc.sync.dma_start(
                out=wv_sb[:], in_=wv_c.rearrange("(a p) h -> p a h", p=128))

            for b in range(B):
                cssn = perb.tile([128, T], f32, tag="cssn")
                nc.sync.dma_start(out=cssn[0:64, :], in_=cosT[b])
                nc.sync.dma_start(out=cssn[64:128, :], in_=sinT[b])
                kT_sb = perb.tile([128, T], f32r, tag="kT")
                v_sb = [perb.tile([128, H + 1], bf16, tag=f"v{si}",
                                  name=f"v{si}") for si in range(NST)]
                for si in range(NST):
                    nc.vector.memset(v_sb[si][:, H:H + 1], 1.0)

                for tcx in range(NTC):
                    tsl = slice(tcx * TC, (tcx + 1) * TC)
                    # ---- projections for this t-chunk ----
                    qps = [ps.tile([128, TC], f32, tag=f"qps{n}",
                                   name=f"qps{n}") for n in range(G)]
                    kps = ps.tile([128, TC], f32, tag="kps")
                    vps = ps.tile([128, TC], f32, tag="vps")
                    for di in range(ND):
                        xt = xs.tile([128, TC], f32r, tag="xt")
                        cblk, dd = di // 4, (di % 4) * 128
                        nc.sync.dma_start(
                            out=xt[:], in_=xg[cblk * B + b, dd:dd + 128, tsl])
                        xtb = xs.tile([128, TC], bf16, tag="xtb")
                        nc.vector.tensor_copy(out=xtb[:], in_=xt[:])
                        st, sp = di == 0, di == ND - 1
                        for n in range(G):
                            nc.tensor.matmul(
                                qps[n][:], wq_sb[n][:, di, :],
                                xt[:], start=st, stop=sp)
                        nc.tensor.matmul(
                            kps[:], wk_sb[:, di, :],
                            xt[:], start=st, stop=sp)
                        nc.tensor.matmul(
                            vps[:], wv_sb[:, di, :], xtb[:], start=st, stop=sp)

                    # ---- RoPE eviction: psum [h, t] -> sbuf ----
                    cs, sn = cssn[0:64, tsl], cssn[64:128, tsl]
                    qT = []
                    for n in range(G):
                        qt = qp.tile([128, TC], f32r, tag=f"q{n}", name=f"q{n}")
                        t1 = rp.tile([64, TC], f32, tag="r1")
                        t2 = rp.tile([64, TC], f32, tag="r2")
                        nc.vector.tensor_mul(t1[:], qps[n][0:64, :], cs)
                        nc.vector.tensor_mul(t2[:], qps[n][64:128, :], sn)
                        nc.vector.tensor_sub(qt[0:64, :], t1[:], t2[:])
                        t3 = rp.tile([64, TC], f32, tag="r3")
                        t4 = rp.tile([64, TC], f32, tag="r4")
                        nc.vector.tensor_mul(t3[:], qps[n][64:128, :], cs)
                        nc.vector.tensor_mul(t4[:], qps[n][0:64, :], sn)
                        nc.vector.tensor_add(qt[64:128, :], t3[:], t4[:])
                        qT.append(qt)
                    t1 = rp.tile([64, TC], f32, tag="r1")
                    t2 = rp.tile([64, TC], f32, tag="r2")
                    nc.vector.tensor_mul(t1[:], kps[0:64, :], cs)
                    nc.vector.tensor_mul(t2[:], kps[64:128, :], sn)
                    nc.vector.tensor_sub(kT_sb[0:64, tsl], t1[:], t2[:])
                    t3 = rp.tile([64, TC], f32, tag="r3")
                    t4 = rp.tile([64, TC], f32, tag="r4")
                    nc.vector.tensor_mul(t3[:], kps[64:128, :], cs)
                    nc.vector.tensor_mul(t4[:], kps[0:64, :], sn)
                    nc.vector.tensor_add(kT_sb[64:128, tsl], t3[:], t4[:])
                    # v: cast + transpose to [s, h] bf16
                    vb = rp.tile([128, TC], bf16, tag="vb")
                    nc.vector.tensor_copy(out=vb[:], in_=vps[:])
                    for j in range(TC // 128):
                        vtp = ps.tile([128, 128], bf16, tag="vps", name="vtp")
                        nc.tensor.transpose(
                            vtp[:], vb[:, j * 128:(j + 1) * 128], ident_b[:])
                        nc.vector.tensor_copy(
                            out=v_sb[tcx * 4 + j][:, 0:H], in_=vtp[:])

                    # ---- attention for this t-chunk ----
                    slist = [si for si in range(NST) if cls[b][tcx][si] != 0]
                    oT = [[None] * (TC // 128) for _ in range(G)]
                    for n in range(G):
                        pts = {}
                        for ii, si in enumerate(slist):
                            pps = ps.tile([128, TC], f32,
                                          tag=f"qps{ii % 2}", name="pps")
                            nc.tensor.matmul(
                                pps[:],
                                kT_sb[:, si * ST:(si + 1) * ST],
                                qT[n][:], start=True, stop=True)
                            ptt = ptp.tile([128, TC], bf16, tag=f"pt{si}",
                                           name=f"pt{si}")
                            nc.scalar.activation(
                                ptt[:], pps[:], AF.Exp, scale=SCALE)
                            if cls[b][tcx][si] == 2:
                                mt = mkp.tile([128, TC], bf16, tag="mk")
                                nc.sync.dma_start(
                                    out=mt[:],
                                    in_=maskP[pidx[(b, tcx, si)]])
                                nc.vector.tensor_mul(ptt[:], ptt[:], mt[:])
                            pts[si] = ptt
                        for ts in range(TC // 128):
                            avp = ps.tile([128, H + 1], f32,
                                          tag=f"qps{2 + ts % 2}", name="avp")
                            for i, si in enumerate(slist):
                                nc.tensor.matmul(
                                    avp[:],
                                    pts[si][:, ts * 128:(ts + 1) * 128],
                                    v_sb[si][:], start=i == 0,
                                    stop=i == len(slist) - 1)
                            rcp = sm.tile([128, 1], f32, tag="rcp")
                            nc.vector.reciprocal(rcp[:], avp[:, H:H + 1])
                            osb = sm.tile([128, 128], bf16, tag="osb")
                            nc.scalar.activation(
                                osb[:], avp[:, 0:H], AF.Copy, scale=rcp[:])
                            otp = ps.tile([128, 128], bf16, tag="kps",
                                          name="otp")
                            nc.tensor.transpose(otp[:], osb[:], ident_b[:])
                            ot = op.tile([128, 128], bf16, tag=f"oT{n}_{ts}",
                                         name=f"oT{n}_{ts}")
                            nc.vector.tensor_copy(out=ot[:], in_=otp[:])
                            oT[n][ts] = ot

                    # ---- o-proj for this t-chunk (wo streamed per dc) ----
                    for dc in range(D // TC):
                        wo_t = []
                        for n in range(G):
                            wt = wop.tile([128, TC], bf16, tag=f"wo{n}",
                                          name=f"wo{n}")
                            nc.sync.dma_start(
                                out=wt[:],
                                in_=wo_c[n][:, dc * TC:(dc + 1) * TC])
                            wo_t.append(wt)
                        for ts in range(TC // 128):
                            ops = ps.tile([128, TC], f32,
                                          tag=("vps", "kps")[dc % 2],
                                          name="ops")
                            for n in range(G):
                                nc.tensor.matmul(
                                    ops[:], oT[n][ts][:], wo_t[n][:],
                                    start=n == 0, stop=n == G - 1)
                            ob = obp.tile([128, TC], f32, tag="ob")
                            nc.vector.tensor_copy(out=ob[:], in_=ops[:])
                            trow = tcx * TC + ts * 128
                            nc.sync.dma_start(
                                out=pout_i[b * T + trow:b * T + trow + 128,
                                           dc * TC:(dc + 1) * TC],
                                in_=ob[:])
            nc.gpsimd.collective_compute(
                "ReduceScatter", mybir.AluOpType.add,
                replica_groups=[list(range(NC))],
                ins=[pout_i.opt()], outs=[rs_out.opt()])
            nc.sync.dma_start(out=pout[:, :], in_=rs_out[:])
    nc.finalize()
    return nc


def kernel(x, segment_pos, attn_mask, wq, wk, wv, wo):
    import ml_dtypes
    from concourse.bass_utils import run_bass_kernel_spmd

    x = np.asarray(x, dtype=np.float32)
    attn_mask = np.asarray(attn_mask).astype(bool)
    bf = ml_dtypes.bfloat16

    # host prep
    xT = np.ascontiguousarray(x.transpose(0, 2, 1))
    pos = np.asarray(segment_pos).astype(np.float32)
    fraction = (2.0 * np.arange(64, dtype=np.float32)) / float(H)
    timescale = (ROPE_THETA ** fraction).astype(np.float32)
    sinusoid = pos[:, :, None] / timescale[None, None, :]  # [B,T,64]
    cosT = np.ascontiguousarray(
        np.cos(sinusoid).astype(np.float32).transpose(0, 2, 1))
    sinT = np.ascontiguousarray(
        np.sin(sinusoid).astype(np.float32).transpose(0, 2, 1))
    cls = _classify(attn_mask)
    parts = [(b, tcx, si) for b in range(B) for tcx in range(NTC)
             for si in range(NST) if cls[b][tcx][si] == 2]
    if parts:
        maskP = np.stack([
            np.ascontiguousarray(
                attn_mask[b, tcx * TC:(tcx + 1) * TC,
                          si * ST:(si + 1) * ST].T).astype(bf)
            for (b, tcx, si) in parts])
    else:
        maskP = np.zeros((1, ST, TC), dtype=bf)
    wq_r = np.ascontiguousarray(
        np.asarray(wq, dtype=np.float32).transpose(1, 0, 2))  # [N,D,H]
    wk_r = np.ascontiguousarray(
        np.asarray(wk, dtype=np.float32).transpose(1, 0, 2))  # [K,D,H]
    wv_r = np.ascontiguousarray(
        np.asarray(wv, dtype=np.float32).transpose(1, 0, 2)).astype(bf)
    wo_b = np.asarray(wo, dtype=np.float32).astype(bf)       # [N,H,D]

    key = str(cls)
    if key not in _CACHE:
        _CACHE[key] = _build(cls)
    nc = _CACHE[key]

    in_maps = []
    for c in range(NC):
        in_maps.append({
            "xsl": np.ascontiguousarray(
                xT[:, c * (D // NC):(c + 1) * (D // NC), :]),
            "cosT": cosT, "sinT": sinT, "maskP": maskP,
            "wq_c": np.ascontiguousarray(wq_r[G * c:G * (c + 1)]),
            "wk_c": np.ascontiguousarray(wk_r[c]),
            "wv_c": np.ascontiguousarray(wv_r[c]),
            "wo_c": np.ascontiguousarray(wo_b[G * c:G * (c + 1)]),
        })
    res = run_bass_kernel_spmd(nc, in_maps, list(range(NC)))
    out = np.concatenate([res.results[c]["pout"] for c in range(NC)], axis=0)
    return np.ascontiguousarray(out.reshape(B, T, D).astype(np.float32))
